# revision 1
# baseline (speedup 1.0000x reference)
"""Distributed attention kernel for Trainium2 (8 NeuronCores).

Problem: non-causal multi-head attention with GQA (16 q heads, 4 kv heads,
head_dim 64, dim 1024, batch 2, seqlen 2048), fp32.

Sharding (per the batch+head hint): core c in 0..7 handles batch b = c//4
and kv-head-group g = c%4 (q heads 4g..4g+3, kv head g). Each core holds the
full sequence, so softmax needs no communication. The output projection is
row-parallel: core (b, g) computes the partial product
O_g @ wo[256g:256(g+1), :] and the host sums the 4 partials per batch
(the gather/unshard step).

Per-core dataflow (activations kept feature-major, scores transposed):
  xT = x[b].T                               (1024, S) fed from host
  QT = wq_g.T @ xT                          (256, S)  [head pair ft: rows
                                              0-63 = head 2ft, 64-127 = 2ft+1]
  KVT = [wk_g | wv_g].T @ xT                (128, S)  [K^T | V^T stacked]
  K^T duplicated to partitions 64-127 so the two heads of a pair run as
  concurrent row-group-tiled matmuls on the PE array.
  V (seq-major, via PE transpose) packed as V''_A = [V | 1], V''_B = [1 | V]:
  the ones block makes the PV matmul produce the softmax denominator
  (replicated across 64 partitions) for free in the same instruction.
  Per (head pair, q-chunk of 512):
    per k-tile (16 x 128): S^T = K^T.T @ Q^T into a fused 2-bank psum tile,
    one exp (ScalarE, psum->sbuf) covering both heads, then PV accumulation
    pv += V''.T @ exp(S^T)   ->  [O ; colsum] in psum.
    normalize: reciprocal on DVE, partition-shift via gpsimd DMA, multiply.
  out rows = OT.T @ wo_g  (activation-stationary matmul), DMA to DRAM.

fp32 data is computed as float32r on the PE (full-rate fp32 matmul mode);
measured end-to-end relative error ~3e-4.
"""

import numpy as np
from contextlib import ExitStack

import concourse.bass as bass
import concourse.mybir as mybir
import concourse.tile as tile
from concourse.bass import ds
from concourse import bass_utils

F32 = mybir.dt.float32
F32R = mybir.dt.float32r

DIM = 1024
N_HEADS = 16
N_KV_HEADS = 4
HD = 64
FH = 256                   # q features per core (4 heads x 64)
KV = 128                   # [K | V] projected feature width per core
D_TILES = DIM // 128       # 8
SEQ = 2048
BSZ = 2
N_CORES = 8


def build_attention_core(nc, S=SEQ, use_f32r=True,
                         sc_bufs=2, pv_bufs=2, exp_bufs=6, acc_bufs=2):
    QCH = 512
    S_TILES = S // 128
    N_QC = S // QCH
    MDT = F32R if use_f32r else F32

    xT = nc.declare_dram_parameter("xT", [DIM, S], MDT, isOutput=False)
    identd = nc.declare_dram_parameter("ident", [128, 128], MDT, isOutput=False)
    onesd = nc.declare_dram_parameter("ones", [128, 64], MDT, isOutput=False)
    wq = nc.declare_dram_parameter("wq", [DIM, FH], MDT, isOutput=False)
    wkv = nc.declare_dram_parameter("wkv", [DIM, KV], MDT, isOutput=False)
    wo = nc.declare_dram_parameter("wo", [FH, DIM], MDT, isOutput=False)
    out = nc.declare_dram_parameter("out", [S, DIM], F32, isOutput=True)

    with tile.TileContext(nc) as tc, ExitStack() as ctx:
        const_p = ctx.enter_context(tc.tile_pool(name="const", bufs=1))
        big_p = ctx.enter_context(tc.tile_pool(name="big", bufs=1))
        exp_p = ctx.enter_context(tc.tile_pool(name="exp", bufs=exp_bufs))
        norm_p = ctx.enter_context(tc.tile_pool(name="norm", bufs=4))
        ps_sc = ctx.enter_context(tc.tile_pool(name="ps_sc", bufs=sc_bufs, space="PSUM"))
        ps_pv = ctx.enter_context(tc.tile_pool(name="ps_pv", bufs=pv_bufs, space="PSUM"))
        ps_acc = ctx.enter_context(tc.tile_pool(name="ps_acc", bufs=acc_bufs, space="PSUM"))

        ident = const_p.tile([128, 128], MDT)
        nc.sync.dma_start(ident[:], identd[:, :])

        # ---- loads (chunked so projections start before the full 8MB) ----
        wq_sb = big_p.tile([128, D_TILES, FH], MDT)
        wkv_sb = big_p.tile([128, D_TILES, KV], MDT)
        for a in range(D_TILES):
            nc.sync.dma_start(wkv_sb[:, a, :], wkv[ds(a * 128, 128), :])
            nc.sync.dma_start(wq_sb[:, a, :], wq[ds(a * 128, 128), :])
        xt_sb = big_p.tile([128, D_TILES, S], MDT)
        for sc in range(N_QC):
            for a in range(D_TILES):
                nc.sync.dma_start(
                    xt_sb[:, a, ds(sc * QCH, QCH)],
                    xT[ds(a * 128, 128), ds(sc * QCH, QCH)],
                )
        wo_sb = big_p.tile([128, 2, DIM], MDT)
        for t in range(2):
            nc.sync.dma_start(wo_sb[:, t, :], wo[ds(t * 128, 128), :])

        # ---- projections --------------------------------------------------
        kvt_sb = big_p.tile([128, S], MDT)
        for sc in range(N_QC):
            acc = ps_acc.tile([128, QCH], F32, tag="acc")
            for a in range(D_TILES):
                nc.tensor.matmul(
                    acc[:], wkv_sb[:, a, :], xt_sb[:, a, ds(sc * QCH, QCH)],
                    start=(a == 0), stop=(a == D_TILES - 1),
                )
            nc.vector.tensor_copy(kvt_sb[:, ds(sc * QCH, QCH)], acc[:])

        qt_sb = big_p.tile([128, 2, S], MDT)
        for sc in range(N_QC):
            for ft in range(2):
                acc = ps_acc.tile([128, QCH], F32, tag="acc")
                for a in range(D_TILES):
                    nc.tensor.matmul(
                        acc[:], wq_sb[:, a, ds(ft * 128, 128)],
                        xt_sb[:, a, ds(sc * QCH, QCH)],
                        start=(a == 0), stop=(a == D_TILES - 1),
                    )
                nc.vector.tensor_copy(qt_sb[:, ft, ds(sc * QCH, QCH)], acc[:])

        # duplicate K^T to partitions 64-127 (head-B row groups)
        kt2_sb = big_p.tile([128, S], MDT)
        nc.gpsimd.dma_start(kt2_sb[64:128, :], kvt_sb[0:64, :])

        # V'' tiles: va = [V | 1], vb = [1 | V]
        va_sb = big_p.tile([128, S_TILES, 128], MDT)
        vb_sb = big_p.tile([128, S_TILES, 128], MDT)
        for kt in range(S_TILES):
            nc.sync.dma_start(va_sb[:, kt, 64:128], onesd[:, :])
            nc.sync.dma_start(vb_sb[:, kt, 0:64], onesd[:, :])
        for kt in range(S_TILES):
            tr = ps_acc.tile([128, 64], MDT, tag="acc")
            nc.tensor.transpose(
                tr[:], kvt_sb[64:128, ds(kt * 128, 128)], ident[64:128, 64:128]
            )
            nc.vector.tensor_copy(va_sb[:, kt, 0:64], tr[:])
            nc.vector.tensor_copy(vb_sb[:, kt, 64:128], tr[:])

        ot_sb = big_p.tile([128, 2, S], MDT)

        # ---- attention + output projection, pipelined per q-chunk --------
        for qc in range(N_QC):
            qsl = ds(qc * QCH, QCH)
            for ft in range(2):
                pva = ps_pv.tile([128, QCH], F32, tag="pv")
                pvb = ps_pv.tile([128, QCH], F32, tag="pv")
                for kt in range(S_TILES):
                    ksl = ds(kt * 128, 128)
                    sc2 = ps_sc.tile([128, 2, QCH], F32, tag="sc")
                    nc.tensor.matmul(
                        sc2[:, 0, :], kvt_sb[0:64, ksl], qt_sb[0:64, ft, qsl],
                        start=True, stop=True,
                    )
                    nc.tensor.matmul(
                        sc2[:, 1, :], kt2_sb[64:128, ksl], qt_sb[64:128, ft, qsl],
                        start=True, stop=True,
                    )
                    e2 = exp_p.tile([128, 2, QCH], MDT, tag="etile")
                    nc.scalar.activation(
                        e2[:, :, :], sc2[:, :, :], mybir.ActivationFunctionType.Exp
                    )
                    nc.tensor.matmul(
                        pva[:], va_sb[:, kt, :], e2[:, 0, :],
                        start=(kt == 0), stop=(kt == S_TILES - 1),
                    )
                    nc.tensor.matmul(
                        pvb[:], vb_sb[:, kt, :], e2[:, 1, :],
                        start=(kt == 0), stop=(kt == S_TILES - 1),
                    )
                # head A: O rows 0-63, colsum (replicated) rows 64-127
                ra = norm_p.tile([128, QCH], F32, tag="ntile")
                nc.vector.reciprocal(ra[64:128, :], pva[64:128, :])
                ra2 = norm_p.tile([128, QCH], F32, tag="ntile")
                nc.gpsimd.dma_start(ra2[0:64, :], ra[64:128, :])
                nc.vector.tensor_mul(ot_sb[0:64, ft, qsl], pva[0:64, :], ra2[0:64, :])
                # head B: colsum rows 0-63, O rows 64-127
                rb = norm_p.tile([128, QCH], F32, tag="ntile")
                nc.vector.reciprocal(rb[0:64, :], pvb[0:64, :])
                rb2 = norm_p.tile([128, QCH], F32, tag="ntile")
                nc.gpsimd.dma_start(rb2[64:128, :], rb[0:64, :])
                nc.vector.tensor_mul(ot_sb[64:128, ft, qsl], pvb[64:128, :], rb2[64:128, :])

            for st in range(QCH // 128):
                row0 = qc * QCH + st * 128
                for c in range(2):
                    acc = ps_acc.tile([128, 512], F32, tag="acc")
                    for ft in range(2):
                        nc.tensor.matmul(
                            acc[:], ot_sb[:, ft, ds(row0, 128)],
                            wo_sb[:, ft, ds(c * 512, 512)],
                            start=(ft == 0), stop=(ft == 1),
                        )
                    stg = norm_p.tile([128, 512], F32, tag="ostg")
                    nc.vector.tensor_copy(stg[:], acc[:])
                    nc.sync.dma_start(out[ds(row0, 128), ds(c * 512, 512)], stg[:])

    return nc


# The neuronx compiler in this environment accepts only ONE sync-wait command
# per instruction; Tile emits instructions with several. Waiting is monotone,
# so hoisting all but the last wait onto same-engine NoOps is equivalent.
_wsctr = [0]


def split_multi_waits(nc):
    n_split = 0
    for f in nc.m.functions:
        for bb in f.blocks:
            insts = bb.instructions
            if not any(
                i.sync_info is not None and len(i.sync_info.on_wait) > 1
                for i in insts
            ):
                continue
            new = []
            for i in insts:
                si = i.sync_info
                if si is not None and len(si.on_wait) > 1:
                    waits = list(si.on_wait)
                    for w in waits[:-1]:
                        _wsctr[0] += 1
                        nop = mybir.InstNoOp(name=f"wsplit_{_wsctr[0]}", ins=[], outs=[])
                        nop.engine = i.engine
                        nop.sync_info = mybir.SyncInfo(on_wait=[w], on_update=[])
                        new.append(nop)
                    i.sync_info = mybir.SyncInfo(
                        on_wait=[waits[-1]], on_update=list(si.on_update)
                    )
                    n_split += 1
                new.append(i)
            bb.instructions = new
    return n_split


def build(use_f32r=True):
    nc = bass.Bass(target_bir_lowering=False)
    build_attention_core(nc, SEQ, use_f32r=use_f32r)
    split_multi_waits(nc)
    return nc


def shard_inputs(x, wq, wk, wv, wo):
    """Full inputs -> per-core in_maps. Core c = (b = c//4, g = c%4)."""
    x = np.asarray(x, np.float32)
    wq = np.asarray(wq, np.float32)
    wk = np.asarray(wk, np.float32)
    wv = np.asarray(wv, np.float32)
    wo = np.asarray(wo, np.float32)
    ident = np.eye(128, dtype=np.float32)
    ones = np.ones((128, 64), np.float32)
    xTs = [np.ascontiguousarray(x[b].T) for b in range(BSZ)]
    in_maps = []
    for c in range(N_CORES):
        b, g = c // 4, c % 4
        # fold the 1/sqrt(head_dim) score scaling into wq
        wq_g = np.ascontiguousarray(wq[:, g * FH:(g + 1) * FH]) * (1.0 / np.sqrt(HD))
        wkv_g = np.ascontiguousarray(
            np.concatenate(
                [wk[:, g * HD:(g + 1) * HD], wv[:, g * HD:(g + 1) * HD]], axis=1
            )
        )
        wo_g = np.ascontiguousarray(wo[g * FH:(g + 1) * FH, :])
        in_maps.append(
            {"xT": xTs[b], "wq": wq_g, "wkv": wkv_g, "wo": wo_g,
             "ident": ident, "ones": ones}
        )
    return in_maps


def unshard_output(results):
    """Sum the 4 row-parallel partial outputs per batch."""
    out = np.zeros((BSZ, SEQ, DIM), np.float32)
    for c in range(N_CORES):
        out[c // 4] += np.asarray(results[c]["out"], np.float32)
    return out


_cache = {}


def kernel(x, wq, wk, wv, wo):
    if "nc" not in _cache:
        _cache["nc"] = build()
    nc = _cache["nc"]
    in_maps = shard_inputs(x, wq, wk, wv, wo)
    res = bass_utils.run_bass_kernel_spmd(nc, in_maps, core_ids=list(range(N_CORES)))
    return unshard_output(res.results)
